# revision 19
# baseline (speedup 1.0000x reference)
"""FP8 blockwise QDQ linear (LumenLinear) on 8 TRN2 NeuronCores. v5

Strategy: tensor-parallel shard along out_features, 1376 columns per
core (11008 = 8*1376). Weight QDQ on host (cf. sharding hint: weight
and its 128x128 block scales are shardable artifacts); dequantized
weight ships as fp16 [K, N/8] over the gpsimd SWDGE queue.

Activation QDQ on device per 128-row m-tile, with m-tiles processed in
PAIRS to halve xbar-transpose mode transitions and DMA call overheads:
  - x loaded with an fp32->fp16 cast during SWDGE DMA (halves SBUF
    footprint; fp16 pre-rounding shifts fp8 decisions only within
    2^-12 of block amax -- measured harmless)
  - DVE: blockwise amax, scale prep, fp8 quantize (TRN e4m3 grid via
    scale = max(amax,eps)/224), dequant of first SPLIT k-blocks
  - ACT: dequant of remaining k-blocks (per-partition scale operand)
    plus PSUM->SBUF evictions
  - one 4 MB xbar transpose per PAIR of m-tiles; fp16 matmuls
    accumulate K=4096 into PSUM f32; bias is added on host
"""

import numpy as np
from contextlib import ExitStack

P = 128
M, K, N_FULL = 8192, 4096, 11008
NCORES = 8
NC_ = N_FULL // NCORES   # 1376 columns per core
KT = K // P              # 32 k-tiles
MT = M // P              # 64 m-tiles
NPAIR = MT // 2          # m-tile pairs
WG = 4                   # k-tiles per weight-load group
NWG = KT // WG           # 8 weight DMA groups
SPLIT = 20               # k-blocks dequantized on DVE; rest on ACT
CHUNKS = [(0, 512), (512, 512), (1024, 352)]  # psum chunks of NC_
FP8_MAX_OCP = 448.0
EPS = 1e-12

_CACHE = {}
LAST_RES = None


def _build():
    import concourse.bass as bass
    import concourse.mybir as mybir
    import concourse.tile as tile
    from concourse.tile import add_dep_helper
    from concourse import bacc

    FP32 = mybir.dt.float32
    FP16 = mybir.dt.float16
    FP8 = mybir.dt.float8e4
    BF16 = mybir.dt.bfloat16

    nc = bacc.Bacc("TRN2", target_bir_lowering=False, debug=False,
                   num_devices=NCORES)
    x_d = nc.dram_tensor("x", [M, K], FP32, kind="ExternalInput").ap()
    wT_h = nc.dram_tensor("wT", [K, NC_], FP16, kind="ExternalInput")
    out_h = nc.dram_tensor("out", [M, NC_], BF16, kind="ExternalOutput")

    with tile.TileContext(nc) as tc, ExitStack() as ctx:
        singles = ctx.enter_context(tc.tile_pool(name="singles", bufs=1))
        xpool = ctx.enter_context(tc.tile_pool(name="xpool", bufs=2))
        xq = ctx.enter_context(tc.tile_pool(name="xq", bufs=2))
        xsc = ctx.enter_context(tc.tile_pool(name="xsc", bufs=2))
        opool = ctx.enter_context(tc.tile_pool(name="opool", bufs=2))
        psum = ctx.enter_context(tc.tile_pool(name="psum", bufs=2, space="PSUM"))

        wd = []
        for g in range(NWG):
            wd.append(singles.tile([P, WG, NC_], FP16, name=f"wd{g}", tag=f"wd{g}"))
        # W rides the sync HWDGE ring (pipelines many transfers; the
        # SWDGE ring only keeps ~2 in flight), ahead of all transposes
        w_insts = []
        for g in range(NWG):
            src = bass.AP(tensor=wT_h, offset=g * WG * P * NC_,
                          ap=[[NC_, P], [P * NC_, WG], [1, NC_]])
            w_insts.append(nc.sync.dma_start(out=wd[g][:], in_=src))

        # first two m-tiles run solo (smaller first transposes -> the PE
        # starts as soon as W and one quant chain land); the rest in pairs
        groups = [[0], [1]] + [[m, m + 1] for m in range(2, MT, 2)]
        for pr, grp in enumerate(groups):
            gn = len(grp)
            xdqP = xq.tile([P, gn, K], FP16, tag="xdqP",
                           padded_shape=[P, 2, K])
            for j, mt in enumerate(grp):
                # fp32 -> fp16 cast during the SWDGE load
                xld = xpool.tile([P, K], FP16, tag="xld")
                nc.gpsimd.dma_start(out=xld[:],
                                    in_=x_d[mt * P:(mt + 1) * P, :])

                xam = xsc.tile([P, KT], FP32, tag="xam")
                nc.vector.tensor_reduce(
                    xam[:], xld[:].rearrange("p (t b) -> p t b", b=P),
                    axis=mybir.AxisListType.X, op=mybir.AluOpType.max,
                    apply_absolute_value=True)
                xt_ = xsc.tile([P, KT], FP32, tag="xt_")
                nc.vector.tensor_scalar_max(xt_[:], xam[:], EPS)
                xd = xsc.tile([P, KT], FP32, tag="xd")
                nc.vector.tensor_scalar_mul(xd[:], xt_[:], 1.0 / 224.0)
                xinv = xsc.tile([P, KT], FP32, tag="xinv")
                nc.vector.reciprocal(xinv[:], xd[:])

                q8 = xq.tile([P, K], FP8, tag="q8")
                xinv_bc = xinv[:].rearrange("p (t o) -> p t o", o=1).broadcast_to([P, KT, P])
                nc.vector.tensor_tensor(
                    out=q8[:].rearrange("p (t b) -> p t b", b=P),
                    in0=xld[:].rearrange("p (t b) -> p t b", b=P),
                    in1=xinv_bc, op=mybir.AluOpType.mult)

                xd_bc = xd[:, 0:SPLIT].rearrange("p (t o) -> p t o", o=1).broadcast_to([P, SPLIT, P])
                nc.vector.tensor_tensor(
                    out=xdqP[:, j, 0:SPLIT * P].rearrange("p (t b) -> p t b", b=P),
                    in0=q8[:, 0:SPLIT * P].rearrange("p (t b) -> p t b", b=P),
                    in1=xd_bc, op=mybir.AluOpType.mult)
                for kb in range(SPLIT, KT):
                    nc.scalar.mul(xdqP[:, j, kb * P:(kb + 1) * P],
                                  q8[:, kb * P:(kb + 1) * P],
                                  xd[:, kb:kb + 1])

            # one xbar transpose per group: [128, gn*4096] -> [128, gn*32, 128]
            xTP = xq.tile([P, gn * KT, P], FP16, tag="xTP",
                          padded_shape=[P, 2 * KT, P])
            t_inst = nc.sync.dma_start_transpose(xTP[:], xdqP[:])
            if pr == 0:
                # keep the weight transfers ahead of the first xbar-mode
                # switch: the transpose waits for every W load
                for wi in w_insts:
                    add_dep_helper(t_inst.ins, wi.ins, sync=False,
                                   reason="first transpose after W loads")

            osbP = opool.tile([P, gn, NC_], BF16, tag="osbP",
                              padded_shape=[P, 2, NC_])
            for j, mt in enumerate(grp):
                for ci, (off, cw) in enumerate(CHUNKS):
                    ps = psum.tile([P, cw], FP32, tag=f"ps{ci}")
                    for kt in range(KT):
                        nc.tensor.matmul(
                            ps[:], xTP[:, j * KT + kt, :],
                            wd[kt // WG][:, kt % WG, off:off + cw],
                            start=(kt == 0), stop=(kt == KT - 1))
                    nc.scalar.copy(osbP[:, j, off:off + cw], ps[:])
            dst = bass.AP(tensor=out_h, offset=grp[0] * P * NC_,
                          ap=[[NC_, P], [P * NC_, gn], [1, NC_]])
            nc.sync.dma_start(dst, osbP[:])

    nc.compile()
    return nc


def _host_weight_qdq(weight):
    """Exact replication of the reference 128x128 blockwise fp8 QDQ."""
    import ml_dtypes

    w = np.ascontiguousarray(weight, dtype=np.float32)
    nb, kb = N_FULL // P, K // P
    wb = w.reshape(nb, P, kb, P)
    amax = np.max(np.abs(wb), axis=(1, 3), keepdims=True)
    scale = np.maximum(amax, EPS) / FP8_MAX_OCP
    q = (wb / scale).astype(ml_dtypes.float8_e4m3fn)
    return (q.astype(np.float32) * scale).reshape(N_FULL, K)


def _ensure_profile_hook_importable():
    """concourse.bass_utils imports antenv.axon_hooks when BASS_TRACE=1;
    some images ship an antenv stub without it. Provide a working hook
    (ctypes against libaxon_pjrt.so) only if the module is missing."""
    try:
        import antenv.axon_hooks  # noqa: F401
        return
    except ImportError:
        pass
    try:
        import contextlib
        import ctypes
        import sys
        import types

        m = types.ModuleType("antenv.axon_hooks")
        m._hook = None

        def set_hook(h):
            m._hook = h

        def get_hook():
            if m._hook is None:
                try:
                    lib = ctypes.CDLL("/opt/axon/libaxon_pjrt.so")
                except OSError:
                    return None
                if not hasattr(lib, "axon_start_nrt_profile"):
                    return None
                lib.axon_start_nrt_profile.argtypes = [
                    ctypes.POINTER(ctypes.c_int64), ctypes.c_size_t]
                lib.axon_start_nrt_profile.restype = ctypes.c_int64
                lib.axon_stop_nrt_profile.argtypes = [ctypes.c_char_p]
                lib.axon_stop_nrt_profile.restype = ctypes.c_int64

                @contextlib.contextmanager
                def _hook(output_dir, device_ids):
                    import jax
                    jax.devices()
                    if device_ids:
                        ids = (ctypes.c_int64 * len(device_ids))(*device_ids)
                        rc = lib.axon_start_nrt_profile(ids, len(device_ids))
                    else:
                        rc = lib.axon_start_nrt_profile(None, 0)
                    if rc != 0:
                        raise RuntimeError(f"axon_start_nrt_profile rc={rc}")
                    try:
                        yield
                    finally:
                        n = lib.axon_stop_nrt_profile(str(output_dir).encode())
                        if n < 0:
                            raise RuntimeError(f"axon_stop_nrt_profile rc={n}")

                m._hook = _hook
            return m._hook

        m.set_axon_ntff_profile_hook = set_hook
        m.get_axon_ntff_profile_hook = get_hook
        sys.modules["antenv.axon_hooks"] = m
    except Exception:
        pass


def kernel(input, weight, bias):
    global LAST_RES
    _ensure_profile_hook_importable()
    from concourse.bass_utils import run_bass_kernel_spmd

    if "nc" not in _CACHE:
        _CACHE["nc"] = _build()
    nc = _CACHE["nc"]

    x = np.ascontiguousarray(input, dtype=np.float32)
    wdqT = _host_weight_qdq(weight).astype(np.float16).T

    in_maps = []
    for c in range(NCORES):
        sl = slice(c * NC_, (c + 1) * NC_)
        in_maps.append({
            "x": x,
            "wT": np.ascontiguousarray(wdqT[:, sl]),
        })

    res = run_bass_kernel_spmd(nc, in_maps, list(range(NCORES)))
    LAST_RES = res
    out = np.concatenate(
        [res.results[c]["out"].astype(np.float32) for c in range(NCORES)], axis=1)
    out = np.ascontiguousarray(out, dtype=np.float32)
    out += np.asarray(bias, dtype=np.float32)[None, :]
    return out


# revision 20
# speedup vs baseline: 1.0300x; 1.0300x over previous
"""FP8 blockwise QDQ linear (LumenLinear) on 8 TRN2 NeuronCores. v5

Strategy: tensor-parallel shard along out_features, 1376 columns per
core (11008 = 8*1376). Weight QDQ on host (cf. sharding hint: weight
and its 128x128 block scales are shardable artifacts); dequantized
weight ships as fp16 [K, N/8] over the gpsimd SWDGE queue.

Activation QDQ on device per 128-row m-tile, with m-tiles processed in
PAIRS to halve xbar-transpose mode transitions and DMA call overheads:
  - x loaded with an fp32->fp16 cast during SWDGE DMA (halves SBUF
    footprint; fp16 pre-rounding shifts fp8 decisions only within
    2^-12 of block amax -- measured harmless)
  - DVE: blockwise amax, scale prep, fp8 quantize (TRN e4m3 grid via
    scale = max(amax,eps)/224), dequant of first SPLIT k-blocks
  - ACT: dequant of remaining k-blocks (per-partition scale operand)
    plus PSUM->SBUF evictions
  - one 4 MB xbar transpose per PAIR of m-tiles; fp16 matmuls
    accumulate K=4096 into PSUM f32; bias is added on host
"""

import numpy as np
from contextlib import ExitStack

P = 128
M, K, N_FULL = 8192, 4096, 11008
NCORES = 8
NC_ = N_FULL // NCORES   # 1376 columns per core
KT = K // P              # 32 k-tiles
MT = M // P              # 64 m-tiles
NPAIR = MT // 2          # m-tile pairs
WG = 4                   # k-tiles per weight-load group
NWG = KT // WG           # 8 weight DMA groups
SPLIT = 20               # k-blocks dequantized on DVE; rest on ACT
CHUNKS = [(0, 512), (512, 512), (1024, 352)]  # psum chunks of NC_
FP8_MAX_OCP = 448.0
EPS = 1e-12

_CACHE = {}
LAST_RES = None


def _build():
    import concourse.bass as bass
    import concourse.mybir as mybir
    import concourse.tile as tile
    from concourse.tile import add_dep_helper
    from concourse import bacc

    FP32 = mybir.dt.float32
    FP16 = mybir.dt.float16
    FP8 = mybir.dt.float8e4
    BF16 = mybir.dt.bfloat16

    nc = bacc.Bacc("TRN2", target_bir_lowering=False, debug=False,
                   num_devices=NCORES)
    x_d = nc.dram_tensor("x", [M, K], FP32, kind="ExternalInput").ap()
    wT_h = nc.dram_tensor("wT", [K, NC_], FP16, kind="ExternalInput")
    out_h = nc.dram_tensor("out", [M, NC_], BF16, kind="ExternalOutput")

    with tile.TileContext(nc) as tc, ExitStack() as ctx:
        singles = ctx.enter_context(tc.tile_pool(name="singles", bufs=1))
        xpool = ctx.enter_context(tc.tile_pool(name="xpool", bufs=2))
        xq = ctx.enter_context(tc.tile_pool(name="xq", bufs=2))
        xsc = ctx.enter_context(tc.tile_pool(name="xsc", bufs=2))
        opool = ctx.enter_context(tc.tile_pool(name="opool", bufs=2))
        psum = ctx.enter_context(tc.tile_pool(name="psum", bufs=2, space="PSUM"))

        wd = []
        for g in range(NWG):
            wd.append(singles.tile([P, WG, NC_], FP16, name=f"wd{g}", tag=f"wd{g}"))
        # W rides the sync HWDGE ring (pipelines many transfers; the
        # SWDGE ring only keeps ~2 in flight), ahead of all transposes
        w_insts = []
        for g in range(NWG):
            src = bass.AP(tensor=wT_h, offset=g * WG * P * NC_,
                          ap=[[NC_, P], [P * NC_, WG], [1, NC_]])
            w_insts.append(nc.sync.dma_start(out=wd[g][:], in_=src))

        # first two m-tiles run solo (smaller first transposes -> the PE
        # starts as soon as W and one quant chain land); the rest in pairs
        groups = [[0], [1]] + [[m, m + 1] for m in range(2, MT, 2)]
        for pr, grp in enumerate(groups):
            gn = len(grp)
            xdqP = xq.tile([P, gn, K], FP16, tag="xdqP",
                           padded_shape=[P, 2, K])
            for j, mt in enumerate(grp):
                # fp32 -> fp16 cast during the SWDGE load
                xld = xpool.tile([P, K], FP16, tag="xld")
                nc.gpsimd.dma_start(out=xld[:],
                                    in_=x_d[mt * P:(mt + 1) * P, :])

                xam = xsc.tile([P, KT], FP32, tag="xam")
                nc.vector.tensor_reduce(
                    xam[:], xld[:].rearrange("p (t b) -> p t b", b=P),
                    axis=mybir.AxisListType.X, op=mybir.AluOpType.max,
                    apply_absolute_value=True)
                xt_ = xsc.tile([P, KT], FP32, tag="xt_")
                nc.vector.tensor_scalar_max(xt_[:], xam[:], EPS)
                xd = xsc.tile([P, KT], FP32, tag="xd")
                nc.vector.tensor_scalar_mul(xd[:], xt_[:], 1.0 / 224.0)
                xinv = xsc.tile([P, KT], FP32, tag="xinv")
                nc.vector.reciprocal(xinv[:], xd[:])

                q8 = xq.tile([P, K], FP8, tag="q8")
                xinv_bc = xinv[:].rearrange("p (t o) -> p t o", o=1).broadcast_to([P, KT, P])
                nc.vector.tensor_tensor(
                    out=q8[:].rearrange("p (t b) -> p t b", b=P),
                    in0=xld[:].rearrange("p (t b) -> p t b", b=P),
                    in1=xinv_bc, op=mybir.AluOpType.mult)

                xd_bc = xd[:, 0:SPLIT].rearrange("p (t o) -> p t o", o=1).broadcast_to([P, SPLIT, P])
                nc.vector.tensor_tensor(
                    out=xdqP[:, j, 0:SPLIT * P].rearrange("p (t b) -> p t b", b=P),
                    in0=q8[:, 0:SPLIT * P].rearrange("p (t b) -> p t b", b=P),
                    in1=xd_bc, op=mybir.AluOpType.mult)
                for kb in range(SPLIT, KT):
                    nc.scalar.mul(xdqP[:, j, kb * P:(kb + 1) * P],
                                  q8[:, kb * P:(kb + 1) * P],
                                  xd[:, kb:kb + 1])

            # one xbar transpose per group: [128, gn*4096] -> [128, gn*32, 128]
            xTP = xq.tile([P, gn * KT, P], FP16, tag="xTP",
                          padded_shape=[P, 2 * KT, P])
            t_inst = nc.sync.dma_start_transpose(xTP[:], xdqP[:])
            if pr == 0:
                # keep the weight transfers ahead of the first xbar-mode
                # switch: the transpose waits for every W load
                for wi in w_insts:
                    add_dep_helper(t_inst.ins, wi.ins, sync=True,
                                   reason="first transpose waits for W loads")

            osbP = opool.tile([P, gn, NC_], BF16, tag="osbP",
                              padded_shape=[P, 2, NC_])
            for j, mt in enumerate(grp):
                for ci, (off, cw) in enumerate(CHUNKS):
                    ps = psum.tile([P, cw], FP32, tag=f"ps{ci}")
                    for kt in range(KT):
                        nc.tensor.matmul(
                            ps[:], xTP[:, j * KT + kt, :],
                            wd[kt // WG][:, kt % WG, off:off + cw],
                            start=(kt == 0), stop=(kt == KT - 1))
                    nc.scalar.copy(osbP[:, j, off:off + cw], ps[:])
            dst = bass.AP(tensor=out_h, offset=grp[0] * P * NC_,
                          ap=[[NC_, P], [P * NC_, gn], [1, NC_]])
            nc.sync.dma_start(dst, osbP[:])

    nc.compile()
    return nc


def _host_weight_qdq(weight):
    """Exact replication of the reference 128x128 blockwise fp8 QDQ."""
    import ml_dtypes

    w = np.ascontiguousarray(weight, dtype=np.float32)
    nb, kb = N_FULL // P, K // P
    wb = w.reshape(nb, P, kb, P)
    amax = np.max(np.abs(wb), axis=(1, 3), keepdims=True)
    scale = np.maximum(amax, EPS) / FP8_MAX_OCP
    q = (wb / scale).astype(ml_dtypes.float8_e4m3fn)
    return (q.astype(np.float32) * scale).reshape(N_FULL, K)


def _ensure_profile_hook_importable():
    """concourse.bass_utils imports antenv.axon_hooks when BASS_TRACE=1;
    some images ship an antenv stub without it. Provide a working hook
    (ctypes against libaxon_pjrt.so) only if the module is missing."""
    try:
        import antenv.axon_hooks  # noqa: F401
        return
    except ImportError:
        pass
    try:
        import contextlib
        import ctypes
        import sys
        import types

        m = types.ModuleType("antenv.axon_hooks")
        m._hook = None

        def set_hook(h):
            m._hook = h

        def get_hook():
            if m._hook is None:
                try:
                    lib = ctypes.CDLL("/opt/axon/libaxon_pjrt.so")
                except OSError:
                    return None
                if not hasattr(lib, "axon_start_nrt_profile"):
                    return None
                lib.axon_start_nrt_profile.argtypes = [
                    ctypes.POINTER(ctypes.c_int64), ctypes.c_size_t]
                lib.axon_start_nrt_profile.restype = ctypes.c_int64
                lib.axon_stop_nrt_profile.argtypes = [ctypes.c_char_p]
                lib.axon_stop_nrt_profile.restype = ctypes.c_int64

                @contextlib.contextmanager
                def _hook(output_dir, device_ids):
                    import jax
                    jax.devices()
                    if device_ids:
                        ids = (ctypes.c_int64 * len(device_ids))(*device_ids)
                        rc = lib.axon_start_nrt_profile(ids, len(device_ids))
                    else:
                        rc = lib.axon_start_nrt_profile(None, 0)
                    if rc != 0:
                        raise RuntimeError(f"axon_start_nrt_profile rc={rc}")
                    try:
                        yield
                    finally:
                        n = lib.axon_stop_nrt_profile(str(output_dir).encode())
                        if n < 0:
                            raise RuntimeError(f"axon_stop_nrt_profile rc={n}")

                m._hook = _hook
            return m._hook

        m.set_axon_ntff_profile_hook = set_hook
        m.get_axon_ntff_profile_hook = get_hook
        sys.modules["antenv.axon_hooks"] = m
    except Exception:
        pass


def kernel(input, weight, bias):
    global LAST_RES
    _ensure_profile_hook_importable()
    from concourse.bass_utils import run_bass_kernel_spmd

    if "nc" not in _CACHE:
        _CACHE["nc"] = _build()
    nc = _CACHE["nc"]

    x = np.ascontiguousarray(input, dtype=np.float32)
    wdqT = _host_weight_qdq(weight).astype(np.float16).T

    in_maps = []
    for c in range(NCORES):
        sl = slice(c * NC_, (c + 1) * NC_)
        in_maps.append({
            "x": x,
            "wT": np.ascontiguousarray(wdqT[:, sl]),
        })

    res = run_bass_kernel_spmd(nc, in_maps, list(range(NCORES)))
    LAST_RES = res
    out = np.concatenate(
        [res.results[c]["out"].astype(np.float32) for c in range(NCORES)], axis=1)
    out = np.ascontiguousarray(out, dtype=np.float32)
    out += np.asarray(bias, dtype=np.float32)[None, :]
    return out
